# revision 25
# baseline (speedup 1.0000x reference)
"""Multi-head attention (softmax over the query axis) on 8 trn2 cores.

Sharding: tensor-parallel over heads — 2 heads per core. Each core computes
its heads' projections + attention + a partial output projection (row-parallel
Wo); the host sums the 8 partial outputs and adds bo.

Device-side layout choices (host pre-packs everything):
  - activations are shipped TRANSPOSED (d on partitions) as bf16, so every
    matmul contracts over the partition dim with natural-layout DMAs.
  - scores are computed transposed ([t, s]) so the softmax axis (query s) is
    the free axis: one exp-activation per strip with fused row-sum.
  - 1/rowsum is folded into V's rows (16K elems) instead of the attention
    matrix (4.2M elems).

v2 schedule: one continuous 32-strip exp stream on the scalar engine (head 0
strips then head 1 strips), with the V projection filling the PE during the
h0 stream and both pass2 accumulations (as 2-bank half-passes) filling it
during the h1 stream. PSUM: scores 2x[128,1024] + 4x[128,512] pass2/V tags
= exactly 8 banks. Bias copies run on the vector engine, weight loads on the
scalar DMA queue, activation loads on sync, stores alternate sync/gpsimd.
"""

import json

import numpy as np
import ml_dtypes

import concourse.bass as bass
import concourse.mybir as mybir
import concourse.tile as tile
from concourse import bass_utils

BF16 = mybir.dt.bfloat16
F32 = mybir.dt.float32
AF = mybir.ActivationFunctionType
ALU = mybir.AluOpType

N_CORES = 8
H = 16
D = 2048
DK = 128
S = 2048
HPC = H // N_CORES          # heads per core = 2
NT = D // 128               # 16 tiles along d / t
NSC = S // 512              # 4 chunks of 512 along s / m
SCALE = 1.0 / float(np.sqrt(DK))
NEXP = 17                   # exp strip buffers (>16 so h1 can run ahead)

TRACE = False
LAST_RESULTS = None
PHASE_MARKS = []


def _mark(nc, label):
    PHASE_MARKS.append((label, nc.next_id()))


# The walrus in this container accepts only ONE sem-wait per instruction
# (setupSyncWait: "Too many sync wait commands"), but Tile attaches one wait
# per depended-on semaphore. Split extra waits onto single-wait NoOps inserted
# just before the instruction on the same engine, at BIR-JSON level so every
# compile path (native + bass2jax/axon) is covered.
def _split_multi_waits(raw: bytes) -> bytes:
    m = json.loads(raw)
    ctr = 0
    changed = False
    for fn in m.get("functions", []):
        for blk in fn.get("blocks", []):
            insts = blk.get("instructions", [])
            out = []
            for inst in insts:
                si = inst.get("sync_info")
                waits = (si.get("on_wait") or []) if si else []
                if len(waits) > 1:
                    changed = True
                    for w in waits[:-1]:
                        ctr += 1
                        out.append(
                            {
                                "debug": inst.get("debug"),
                                "engine": inst["engine"],
                                "ins": [],
                                "name": f"I-wsplit-{ctr}",
                                "opcode": "NoOp",
                                "outs": [],
                                "sync_info": {"on_update": [], "on_wait": [w]},
                            }
                        )
                    si["on_wait"] = [waits[-1]]
                out.append(inst)
            if changed:
                blk["instructions"] = out
    if not changed:
        return raw
    return json.dumps(m).encode()


_orig_to_json_bytes = bass.Bass.to_json_bytes


def _to_json_bytes_split(self):
    return _split_multi_waits(_orig_to_json_bytes(self))


bass.Bass.to_json_bytes = _to_json_bytes_split


def _build_bass(loop_n=None):
    nc = bass.Bass(trn_type="TRN2")

    qT = nc.dram_tensor("qT", [D, S], BF16, kind="ExternalInput")
    kT = nc.dram_tensor("kT", [D, S], BF16, kind="ExternalInput")
    vT = nc.dram_tensor("vT", [D, S], BF16, kind="ExternalInput")
    # wq/wk packed dt-major: col = (dt*HPC + h)*128 + k
    wq = nc.dram_tensor("wq", [128, NT * HPC * 128], BF16, kind="ExternalInput")
    wk = nc.dram_tensor("wk", [128, NT * HPC * 128], BF16, kind="ExternalInput")
    wv2 = nc.dram_tensor("wv2", [128, NT * HPC * 128], BF16, kind="ExternalInput")
    wo = nc.dram_tensor("wo", [128, HPC * D], BF16, kind="ExternalInput")
    bqk = nc.dram_tensor("bqk", [128, 2 * HPC], F32, kind="ExternalInput")
    bvb = nc.dram_tensor("bvb", [128, HPC * 128], F32, kind="ExternalInput")
    out = nc.dram_tensor("out_p", [S, D], BF16, kind="ExternalOutput")

    with tile.TileContext(nc) as tc:
        with (
            tc.tile_pool(name="wpool", bufs=1) as wpool,
            tc.tile_pool(name="acts", bufs=1) as acts,
            tc.tile_pool(name="xpool", bufs=4) as xpool,
            tc.tile_pool(name="small", bufs=2) as small,
            tc.tile_pool(name="opool", bufs=2) as opool,
            tc.tile_pool(name="exppool", bufs=1) as exppool,
        ):
            # --- resident weights ---
            wq_sb = wpool.tile([128, NT * HPC * 128], BF16)
            wk_sb = wpool.tile([128, NT * HPC * 128], BF16)
            wv2_sb = wpool.tile([128, NT * HPC * 128], BF16)
            wo_sb = wpool.tile([128, HPC * D], BF16)
            bqk_sb = wpool.tile([128, 2 * HPC], F32)
            bvb_sb = wpool.tile([128, HPC * 128], F32)
            # first dt block lands first (64KB), rest streams behind; all
            # weights ride the scalar engine's DMA queue so the sync queue
            # carries only activation strips.
            nc.scalar.dma_start(wq_sb[:, : HPC * 128], wq[:, : HPC * 128])
            nc.scalar.dma_start(bqk_sb[:], bqk[:])
            nc.scalar.dma_start(wq_sb[:, HPC * 128 :], wq[:, HPC * 128 :])

            # --- resident per-head activations ---
            QT = [acts.tile([128, S], BF16, name=f"QT{h}") for h in range(HPC)]
            KT = [acts.tile([128, S], BF16, name=f"KT{h}") for h in range(HPC)]
            V = [acts.tile([128, NT * 128], BF16, name=f"V{h}") for h in range(HPC)]
            HT = [acts.tile([128, S], BF16, name=f"HT{h}") for h in range(HPC)]

            # benchmark mode: run the whole body loop_n times in one NEFF to
            # amortize dispatch overhead; weights loaded once up front.
            loop_ctx = None
            if loop_n:
                nc.scalar.dma_start(wk_sb[:], wk[:])
                nc.scalar.dma_start(wv2_sb[:], wv2[:])
                nc.scalar.dma_start(bvb_sb[:], bvb[:])
                nc.scalar.dma_start(wo_sb[:], wo[:])
                loop_ctx = tc.For_i(0, loop_n, 1)
                loop_ctx.__enter__()

            # ---------------- phase P-QK: Q^T / K^T projections ---------------
            # Full-strip loads ([128, S] = 4KB lines); one psum bank per
            # (head, s-chunk), accumulated across all 16 d-strips. Bias copies
            # (psum -> sbuf bf16) run on the vector engine so the scalar
            # engine stays clear for the exp stream.
            with tc.tile_pool(name="ppqk", bufs=1, space="PSUM") as ppqk:
                for xdram, w_sb, dst, bcol in ((qT, wq_sb, QT, 0), (kT, wk_sb, KT, HPC)):
                    _mark(nc, "P-Q" if xdram is qT else "P-K")
                    if xdram is kT and not loop_n:
                        nc.scalar.dma_start(wk_sb[:], wk[:])
                    ps = [
                        [
                            ppqk.tile(
                                [128, 512], F32, name=f"pp{h}{c}", tag=f"pp{h}{c}", bufs=1
                            )
                            for c in range(NSC)
                        ]
                        for h in range(HPC)
                    ]
                    for dt in range(NT):
                        if xdram is qT and dt == 0:
                            # split the very first strip into 4 separate
                            # tiles so matmul 0 starts after 128KB, not 512KB
                            xq = []
                            for c4 in range(4):
                                t = xpool.tile(
                                    [128, 512], BF16, name=f"xs0q{c4}", tag=f"xs0q{c4}", bufs=1
                                )
                                nc.sync.dma_start(t[:], xdram[:128, c4 * 512 : (c4 + 1) * 512])
                                xq.append(t)
                            chunk = lambda c: xq[c][:]
                        else:
                            xs = xpool.tile([128, S], BF16, name="xs", tag="xs", bufs=6)
                            nc.sync.dma_start(xs[:], xdram[dt * 128 : (dt + 1) * 128, :])
                            chunk = lambda c: xs[:, c * 512 : (c + 1) * 512]
                        if xdram is qT and dt == 1 and not loop_n:
                            nc.scalar.dma_start(wv2_sb[:], wv2[:])
                            nc.scalar.dma_start(bvb_sb[:], bvb[:])
                        for h in range(HPC):
                            for c in range(NSC):
                                nc.tensor.matmul(
                                    ps[h][c][:],
                                    w_sb[:, (dt * HPC + h) * 128 : (dt * HPC + h + 1) * 128],
                                    chunk(c),
                                    start=(dt == 0),
                                    stop=(dt == NT - 1),
                                )
                    # psum -> sbuf with per-head bias, alternating DVE/scalar
                    # so the two banks gating the next phase drain in parallel
                    # (gpsimd cannot read PSUM).
                    for n, (h, c) in enumerate(
                        [(h, c) for h in range(HPC) for c in range(NSC)]
                    ):
                        if n % 2 == 0:
                            nc.vector.tensor_scalar_add(
                                dst[h][:, c * 512 : (c + 1) * 512],
                                ps[h][c][:],
                                bqk_sb[:, bcol + h : bcol + h + 1],
                            )
                        else:
                            nc.scalar.activation(
                                dst[h][:, c * 512 : (c + 1) * 512],
                                ps[h][c][:],
                                AF.Identity,
                                bias=bqk_sb[:, bcol + h : bcol + h + 1],
                                scale=1.0,
                            )

            # ------------- S section: exp stream + V + pass2 ------------------
            # One continuous 32-strip exp stream on ACT (h0 strips 0..15 then
            # h1 strips 0..15). PE fillers: V projection during the h0 half,
            # pass2 half-passes (2 psum banks each) during the h1 half.
            with tc.tile_pool(name="pps", bufs=1, space="PSUM") as pps:
                if not loop_n:
                    nc.scalar.dma_start(wo_sb[:], wo[:])
                vsca = [
                    small.tile([128, NT * 128], BF16, name=f"vsca{h}", tag=f"vsca{h}", bufs=1)
                    for h in range(HPC)
                ]
                expts = {}
                rects = {}

                def exp_tile(h, i):
                    idx = (h * NT + i) % NEXP
                    t = exppool.tile([128, S], BF16, name=f"expt{idx}", tag=f"exp{idx}", bufs=1)
                    expts[(h, i)] = t
                    return t

                def emit_pass1(h, i):
                    # scores -> exp (+row-sum) -> 1/rowsum; psc half-tiles
                    # [128,1024] = 2 banks each, 2 tags = 1-strip pipeline
                    expt = exp_tile(h, i)
                    sumt = small.tile([128, 2], F32, name="sumt", tag="sum", bufs=4)
                    for half in range(2):
                        psc = pps.tile(
                            [128, 1024], F32, name=f"psc{half}", tag=f"psc{half}", bufs=1
                        )
                        for cc in range(2):
                            c = half * 2 + cc
                            nc.tensor.matmul(
                                psc[:, cc * 512 : (cc + 1) * 512],
                                KT[h][:, i * 128 : (i + 1) * 128],
                                QT[h][:, c * 512 : (c + 1) * 512],
                                start=True,
                                stop=True,
                            )
                        nc.scalar.activation(
                            expt[:, half * 1024 : (half + 1) * 1024],
                            psc[:],
                            AF.Exp,
                            scale=SCALE,
                            accum_out=sumt[:, half : half + 1],
                        )
                    rect = small.tile(
                        [128, 1], F32, name="rect", tag=f"rec{i % 4}", bufs=2
                    )
                    rects[(h, i)] = rect
                    nc.vector.reduce_sum(rect[:], sumt[:], axis=mybir.AxisListType.X)
                    nc.vector.reciprocal(rect[:], rect[:])

                def emit_vsca(h, i):
                    nc.vector.tensor_scalar_mul(
                        vsca[h][:, i * 128 : (i + 1) * 128],
                        V[h][:, i * 128 : (i + 1) * 128],
                        rects[(h, i)][:],
                    )

                # V projection: 16 t-units (one [128,256] psum each,
                # accumulated over 16 dt). One unit per h0 stream slot; psv
                # units alternate between two of the pass2 bank tags.
                xcs = {}

                def load_v_group(tg):
                    for dt in range(NT):
                        xc = xpool.tile([128, 512], BF16, name="xc", tag="xc", bufs=24)
                        nc.sync.dma_start(
                            xc[:], vT[dt * 128 : (dt + 1) * 128, tg * 512 : (tg + 1) * 512]
                        )
                        xcs[(tg, dt)] = xc

                def emit_v_unit(u):
                    tg = u // 4
                    psv = pps.tile(
                        [128, HPC * 128], F32,
                        name=f"psv{u}", tag=("ph00" if u % 2 == 0 else "ph10"), bufs=1,
                    )
                    if u % 4 == 0 and tg + 1 < 4:
                        load_v_group(tg + 1)  # prefetch one group ahead
                    tt = u % 4  # position within the xc chunk
                    for dt in range(NT):
                        nc.tensor.matmul(
                            psv[:],
                            xcs[(tg, dt)][:, tt * 128 : (tt + 1) * 128],
                            wv2_sb[:, (dt * HPC) * 128 : (dt * HPC + HPC) * 128],
                            start=(dt == 0),
                            stop=(dt == NT - 1),
                        )
                    for h in range(HPC):
                        nc.vector.tensor_tensor(
                            V[h][:, u * 128 : (u + 1) * 128],
                            psv[:, h * 128 : (h + 1) * 128],
                            bvb_sb[:, h * 128 : (h + 1) * 128],
                            op=ALU.add,
                        )

                _mark(nc, "S-h0")
                # h0 half: exp stream + all 256 V matmuls as PE filler
                load_v_group(0)
                for i in range(NT):
                    emit_pass1(0, i)
                    emit_v_unit(i)
                    emit_vsca(0, i)

                _mark(nc, "S-h1")
                # h1 half: exp stream + pass2 as PE filler. Both half-passes
                # of a head advance strip-by-strip together (so each exp
                # strip is fully consumed as early as possible, freeing its
                # buffer for the h1 stream): head 0 drains in slots 0..7,
                # head 1 trails its exp stream in slots 8..15.
                def pass2_steps(h):
                    ph = {
                        (half, cc): pps.tile(
                            [128, 512], F32, name=f"ph{half}{cc}", tag=f"ph{half}{cc}", bufs=1
                        )
                        for half in range(2)
                        for cc in range(2)
                    }
                    for i in range(NT):
                        for half in range(2):
                            for cc in range(2):
                                c = half * 2 + cc
                                yield 1
                                nc.tensor.matmul(
                                    ph[(half, cc)][:],
                                    vsca[h][:, i * 128 : (i + 1) * 128],
                                    expts[(h, i)][:, c * 512 : (c + 1) * 512],
                                    start=(i == 0),
                                    stop=(i == NT - 1),
                                )
                    for half in range(2):
                        for cc in range(2):
                            c = half * 2 + cc
                            nc.vector.tensor_copy(
                                HT[h][:, c * 512 : (c + 1) * 512], ph[(half, cc)][:]
                            )

                p2q = [pass2_steps(0), pass2_steps(1)]

                def p2_fill(n):
                    done = 0
                    while done < n and p2q:
                        if next(p2q[0], None) is None:
                            p2q.pop(0)
                        else:
                            done += 1

                for i in range(NT):
                    emit_pass1(1, i)
                    emit_vsca(1, i)
                    p2_fill(8)
                p2_fill(10**9)

            # ---------------- phase O: partial output projection --------------
            # Baseline-proven pipeline: po [128,1024] x3 bufs, ot x4 bufs,
            # 256KB stores on the sync queue; copies alternate vector/scalar.
            # lhsT (HT chunk) held constant across each head's 2 matmuls.
            with tc.tile_pool(name="ppo", bufs=3, space="PSUM") as ppo:
                _mark(nc, "O")
                for st in range(NT):
                    for cp in range(2):
                        po = ppo.tile([128, 1024], F32, name="po", tag="po", bufs=3)
                        for h in range(HPC):
                            for cc in range(2):
                                c = cp * 2 + cc
                                nc.tensor.matmul(
                                    po[:, cc * 512 : (cc + 1) * 512],
                                    HT[h][:, st * 128 : (st + 1) * 128],
                                    wo_sb[:, h * D + c * 512 : h * D + (c + 1) * 512],
                                    start=(h == 0),
                                    stop=(h == HPC - 1),
                                )
                        ot = opool.tile([128, 1024], BF16, name="ot", tag="ot", bufs=5)
                        if cp % 2 == 0:
                            nc.vector.tensor_copy(ot[:], po[:])
                        else:
                            nc.scalar.copy(ot[:], po[:])
                        seng = nc.sync if cp % 2 == 0 else nc.scalar
                        seng.dma_start(
                            out[st * 128 : (st + 1) * 128, cp * 1024 : (cp + 1) * 1024],
                            ot[:],
                        )

            if loop_ctx is not None:
                loop_ctx.__exit__(None, None, None)

    return nc


_NC = None


def _get_nc():
    global _NC
    if _NC is None:
        _NC = _build_bass()
    return _NC


def _prep_inputs(query, key, value, Wq, bq, Wk, bk, Wv, bv, Wo, bo):
    """Host-side shard + pack. Returns per-core input maps."""
    bf = ml_dtypes.bfloat16
    f32 = np.float32

    query = np.asarray(query, f32)
    key = np.asarray(key, f32)
    value = np.asarray(value, f32)
    Wq = np.asarray(Wq, f32)
    Wk = np.asarray(Wk, f32)
    Wv = np.asarray(Wv, f32)
    Wo = np.asarray(Wo, f32)
    bq = np.asarray(bq, f32)
    bk = np.asarray(bk, f32)
    bv = np.asarray(bv, f32)

    qT = np.ascontiguousarray(query.T).astype(bf)
    kT = np.ascontiguousarray(key.T).astype(bf)
    vT = np.ascontiguousarray(value.T).astype(bf)

    in_maps = []
    for c in range(N_CORES):
        heads = [c * HPC + j for j in range(HPC)]

        # wq/wk: [128, NT*HPC*128], col = (dt*HPC + h)*128 + k (dt-major)
        def pack_w(W):
            # W[hh]: [D, DK] -> [NT, 128, DK]; stack heads per dt block
            per_head = [W[hh].reshape(NT, 128, DK) for hh in heads]
            blocks = [
                np.concatenate([ph[dt] for ph in per_head], axis=1)
                for dt in range(NT)
            ]
            return np.concatenate(blocks, axis=1).astype(bf)

        # wv2: [128, NT*HPC*128], col = dt*(HPC*128) + h*128 + k
        wv2 = np.concatenate(
            [
                np.concatenate([Wv[hh].reshape(NT, 128, DK)[dt] for hh in heads], axis=1)
                for dt in range(NT)
            ],
            axis=1,
        ).astype(bf)

        wo_p = np.concatenate(
            [Wo[hh * DK : (hh + 1) * DK, :] for hh in heads], axis=1
        ).astype(bf)

        bqk = np.stack(
            [bq[hh] for hh in heads] + [bk[hh] for hh in heads], axis=1
        ).astype(f32)
        bvb = np.concatenate(
            [np.broadcast_to(bv[hh][None, :], (128, DK)) for hh in heads], axis=1
        ).astype(f32)

        in_maps.append(
            {
                "qT": qT,
                "kT": kT,
                "vT": vT,
                "wq": pack_w(Wq),
                "wk": pack_w(Wk),
                "wv2": np.ascontiguousarray(wv2),
                "wo": np.ascontiguousarray(wo_p),
                "bqk": np.ascontiguousarray(bqk),
                "bvb": np.ascontiguousarray(bvb),
            }
        )
    return in_maps


def kernel(query, key, value, Wq, bq, Wk, bk, Wv, bv, Wo, bo):
    global LAST_RESULTS
    in_maps = _prep_inputs(query, key, value, Wq, bq, Wk, bk, Wv, bv, Wo, bo)
    nc = _get_nc()
    res = bass_utils.run_bass_kernel_spmd(
        nc, in_maps, core_ids=list(range(N_CORES)), trace=TRACE
    )
    LAST_RESULTS = res
    acc = res.results[0]["out_p"].astype(np.float32)
    for c in range(1, N_CORES):
        acc += res.results[c]["out_p"].astype(np.float32)
    acc += np.asarray(bo, np.float32)[None, :]
    return acc
